# revision 20
# baseline (speedup 1.0000x reference)
"""LIF (leaky integrate-and-fire) scan kernel for Trainium2, 8 NeuronCores.

Reference semantics (fp32, T=8 innermost axis):
    mem = 0
    for t in range(T):
        mem = mem * 0.5 + x[..., t]
        s[..., t] = (mem >= 1.0)
        mem = mem * (1.0 - s[..., t])

Strategy (fp32 baseline ~218 us; v3 ~126 us; v4 ~109 us):
  * int16 fixed-point (scale 2^12): x quantized on the host, membrane M
    int16 on device. Engines compute fp32 internally, so the only error
    sources are x quantization and one int16 rounding of M per step
    (quantum 2^-12): 1841 spike flips vs the fp32 reference = rel err
    0.0137 < 2e-2 (robust to HW rounding mode; rne/trunc/floor/ceil all
    pass in host simulation).
  * HW-measured op selection (per 2048 elems/partition):
    scalar_tensor_tensor is ALWAYS 1x (2284 ns) regardless of dtype;
    tensor_tensor on 16-bit is 2x (1225 ns); tensor_scalar 16-bit in/out
    is 4x (694 ns); the mask must be fp16 (int16-out is_lt is
    pathological at 3529 ns, bf16 mixed-TT drops to 1628 ns). The update
    avoids stt entirely and fuses decay+reset into one tensor_tensor via
    a {0, 0.5}-valued mask:
        A: M_t  = tt_add(M'_{t-1}, y_t)            int16, 2x   (y = x*4096)
        B: r2_t = ts(M_t is_lt 4096) mult 0.5      fp16 {0,.5}, 4x
        D: M'_t = tt_mult(M_t, r2_t)               int16, 2x = reset AND decay
    (DMA-CCE accumulate was tried for A: exact for int16, but its RMW
    doubles SDMA port traffic and its ~5.5 us round-trip needs >=4
    in-flight chunk chains to hide, which PSUM (8 banks = 2 chunks of
    packing state) cannot support — measured 127-263 us. Pure-DVE wins.)
  * Packed u8 output (32x less output traffic): PE accumulates 8 matmuls
    with diagonal fp16 weights -2^(t+1) over the r2 planes into PSUM:
    psum = -sum_t 2^t r_t = packed_byte - 255; ACT adds 255 during the
    PSUM -> SBUF u8 copy. byte bit t = spike at step t.
  * Per-core HBM traffic: 16 MiB in + 1 MiB out (~50 us at 358 GB/s).
  * Uneven chunks [4096, 2048, 2048]: the big chunks amortize DVE
    per-instruction overhead; the final 2048 chunk halves the kernel tail
    (last mask -> 4 matmuls -> copies -> out-DMA). Input arrives as
    per-timestep 0.5-1 MiB DMAs, and the t=0/t=1 strips of chunk 0 are
    issued first so compute starts ~3 us in.

Per-core layout: data-parallel over the leading dim (64 -> 8 per core),
t-major strips [128 partitions, T=8, 8192 neurons]; all compute touches
contiguous strips (keeps DVE 2x/4x modes + dense DMA descriptors).
"""

import numpy as np

import concourse.bass as bass
import concourse.tile as tile
from concourse import bacc, mybir
from concourse.bass_utils import run_bass_kernel_spmd

P = 128          # SBUF partitions
T = 8            # timesteps (innermost axis of the original input)
NPB = 8192       # neurons per partition per core: 8*128*32*32 / 128
CHUNKS = (4096, 4096)
PSB = 512        # psum bank free size (fp32)

SCALE = 4096.0   # fixed-point scale 2^12
THR = 4096.0     # threshold 1.0 in scaled units
N_CORES = 8

F32 = mybir.dt.float32
I16 = mybir.dt.int16
U8 = mybir.dt.uint8
F16 = mybir.dt.float16

Alu = mybir.AluOpType
Act = mybir.ActivationFunctionType


def _build() -> bass.Bass:
    nc = bacc.Bacc("TRN2", target_bir_lowering=False, debug=False)
    x = nc.dram_tensor("x", [P, T, NPB], I16, kind="ExternalInput").ap()
    w = nc.dram_tensor("w", [P, T * P], F16, kind="ExternalInput").ap()
    y = nc.dram_tensor("y", [P, NPB], U8, kind="ExternalOutput").ap()

    with tile.TileContext(nc) as tc:
        with (
            tc.tile_pool(name="xin", bufs=9) as xin,
            tc.tile_pool(name="mem", bufs=6) as mem,
            tc.tile_pool(name="msk", bufs=6) as msk,
            tc.tile_pool(name="acc", bufs=2) as accp,
            tc.tile_pool(name="wts", bufs=1) as wts,
            tc.tile_pool(name="ps", bufs=1, space="PSUM") as psp,
        ):
            wt = wts.tile([P, T * P], F16, tag="w", name="wt")
            nc.sync.dma_start(wt[:], w[:, :])
            bias_p = wts.tile([P, 1], F32, tag="bp", name="bias_p")
            nc.vector.memset(bias_p[:], 255.0)

            HH = 2048  # half width for the edge-step splits
            x0h = [
                xin.tile([P, HH], I16, tag="x0h", name=f"x0h{h}", bufs=2)
                for h in range(2)
            ]
            for h in range(2):
                nc.sync.dma_start(x0h[h][:], x[:, 0, h * HH : (h + 1) * HH])
            lo = 0
            bank0 = 0
            for c, CH in enumerate(CHUNKS):
                NB = CH // PSB
                last = c == len(CHUNKS) - 1
                xs = []
                for t in range(T):
                    if c == 0 and t == 0:
                        xs.append(None)  # halves in x0h
                        continue
                    st = xin.tile([P, CH], I16, tag="x", name=f"x{c}_{t}")
                    nc.sync.dma_start(st[:], x[:, t, lo : lo + CH])
                    xs.append(st[:])
                ps = [
                    psp.tile(
                        [P, PSB], F32, tag=f"ps{(bank0 + b) % 8}",
                        name=f"ps{c}_{b}",
                    )
                    for b in range(NB)
                ]
                acc = accp.tile([P, CH], U8, tag="a", name=f"a{c}")
                cur = xs[0]  # M_0 = y_0 (mem starts at 0): alias, no copy
                for t in range(T):
                    if c == 0 and t == 0:
                        # Edge split: each half's mask/reset starts as soon
                        # as its 0.5 MiB strip half lands.
                        r2h = [
                            msk.tile([P, HH], F16, tag="rh", name=f"rh{h}", bufs=2)
                            for h in range(2)
                        ]
                        rst = mem.tile([P, CH], I16, tag="m", name="d0_0")
                        for h in range(2):
                            nc.vector.tensor_scalar(
                                r2h[h][:], x0h[h][:], THR, 0.5, Alu.is_lt, Alu.mult
                            )
                            for b in range(4 * h, 4 * h + 4):
                                nc.tensor.matmul(
                                    ps[b][:],
                                    wt[:, 0:P],
                                    r2h[h][:, (b - 4 * h) * PSB : (b - 4 * h + 1) * PSB],
                                    start=True,
                                    stop=False,
                                )
                            nc.vector.tensor_tensor(
                                rst[:, h * HH : (h + 1) * HH],
                                x0h[h][:],
                                r2h[h][:],
                                Alu.mult,
                            )
                        prev = rst
                        continue
                    if last and t == T - 1:
                        # Edge split: first half's matmuls+copies launch while
                        # the second half's add/mask still runs on the DVE.
                        m7h = [
                            mem.tile([P, HH], I16, tag="mh", name=f"m7h{h}", bufs=2)
                            for h in range(2)
                        ]
                        for h in range(2):
                            nc.vector.tensor_tensor(
                                m7h[h][:],
                                prev[:, h * HH : (h + 1) * HH],
                                xs[t][:, h * HH : (h + 1) * HH],
                                Alu.add,
                            )
                            r2x = msk.tile(
                                [P, HH], F16, tag="rh", name=f"r7h{h}", bufs=2
                            )
                            nc.vector.tensor_scalar(
                                r2x[:], m7h[h][:], THR, 0.5, Alu.is_lt, Alu.mult
                            )
                            for b in range(4 * h, 4 * h + 4):
                                nc.tensor.matmul(
                                    ps[b][:],
                                    wt[:, t * P : (t + 1) * P],
                                    r2x[:, (b - 4 * h) * PSB : (b - 4 * h + 1) * PSB],
                                    start=False,
                                    stop=True,
                                )
                        continue
                    if t > 0:
                        nxt = mem.tile([P, CH], I16, tag="m", name=f"m{c}_{t}")
                        nc.vector.tensor_tensor(nxt[:], prev[:], xs[t], Alu.add)
                        cur = nxt[:]
                    r2 = msk.tile([P, CH], F16, tag="r", name=f"r{c}_{t}")
                    nc.vector.tensor_scalar(
                        r2[:], cur, THR, 0.5, Alu.is_lt, Alu.mult
                    )
                    for b in range(NB):
                        nc.tensor.matmul(
                            ps[b][:],
                            wt[:, t * P : (t + 1) * P],
                            r2[:, b * PSB : (b + 1) * PSB],
                            start=(t == 0),
                            stop=(t == T - 1),
                        )
                    if t < T - 1:
                        rst = mem.tile([P, CH], I16, tag="m", name=f"d{c}_{t}")
                        nc.vector.tensor_tensor(rst[:], cur, r2[:], Alu.mult)
                        prev = rst
                for b in range(NB):
                    nc.scalar.activation(
                        acc[:, b * PSB : (b + 1) * PSB],
                        ps[b][:],
                        Act.Identity,
                        bias=bias_p[:],
                        scale=1.0,
                    )
                    if b % 4 == 3:
                        nc.sync.dma_start(
                            y[:, lo + (b - 3) * PSB : lo + (b + 1) * PSB],
                            acc[:, (b - 3) * PSB : (b + 1) * PSB],
                        )
                lo += CH
                bank0 += NB
    nc.compile()
    return nc


_NC_CACHE: bass.Bass | None = None


def _get_nc() -> bass.Bass:
    global _NC_CACHE
    if _NC_CACHE is None:
        _NC_CACHE = _build()
    return _NC_CACHE


def _weights() -> np.ndarray:
    # W_t = -2^(t+1) * I, laid out as [P, T*P] (lhsT slices [128, 128] per t).
    # psum = sum_t W_t^T r2_t = -sum_t 2^t r_t = packed_byte - 255.
    wf = np.zeros((P, T * P), dtype=np.float32)
    for t in range(T):
        wf[:, t * P : (t + 1) * P][np.arange(P), np.arange(P)] = -(2.0 ** (t + 1))
    return wf.astype(np.float16)


def _run(X: np.ndarray, **spmd_kwargs):
    assert X.shape == (64, 128, 32, 32, 8), X.shape
    X = np.asarray(X, dtype=np.float32)
    per_core = 64 // N_CORES
    q = np.clip(np.rint(X * SCALE), -32768.0, 32767.0).astype(np.int16)
    # [core, p, n, t] -> t-major [core, p, t, n], contiguous per core
    qt = np.ascontiguousarray(
        q.reshape(N_CORES, P, NPB, T).transpose(0, 1, 3, 2)
    )
    wnp = _weights()
    in_maps = [{"x": qt[i], "w": wnp} for i in range(N_CORES)]
    res = run_bass_kernel_spmd(
        _get_nc(), in_maps, core_ids=list(range(N_CORES)), **spmd_kwargs
    )
    out = np.empty_like(X)
    for i, r in enumerate(res.results):
        packed = r["y"].reshape(P, NPB, 1).astype(np.uint8)
        bits = np.unpackbits(packed, axis=2, bitorder="little")  # [P, NPB, 8]
        out[i * per_core : (i + 1) * per_core] = bits.astype(np.float32).reshape(
            per_core, 128, 32, 32, 8
        )
    return out, res


def kernel(X: np.ndarray) -> np.ndarray:
    out, _ = _run(X)
    return out


# revision 21
# speedup vs baseline: 1.0055x; 1.0055x over previous
"""LIF (leaky integrate-and-fire) scan kernel for Trainium2, 8 NeuronCores.

Reference semantics (fp32, T=8 innermost axis):
    mem = 0
    for t in range(T):
        mem = mem * 0.5 + x[..., t]
        s[..., t] = (mem >= 1.0)
        mem = mem * (1.0 - s[..., t])

Strategy (fp32 baseline ~218 us; v3 ~126 us; v4 ~109 us):
  * int16 fixed-point (scale 2^12): x quantized on the host, membrane M
    int16 on device. Engines compute fp32 internally, so the only error
    sources are x quantization and one int16 rounding of M per step
    (quantum 2^-12): 1841 spike flips vs the fp32 reference = rel err
    0.0137 < 2e-2 (robust to HW rounding mode; rne/trunc/floor/ceil all
    pass in host simulation).
  * HW-measured op selection (per 2048 elems/partition):
    scalar_tensor_tensor is ALWAYS 1x (2284 ns) regardless of dtype;
    tensor_tensor on 16-bit is 2x (1225 ns); tensor_scalar 16-bit in/out
    is 4x (694 ns); the mask must be fp16 (int16-out is_lt is
    pathological at 3529 ns, bf16 mixed-TT drops to 1628 ns). The update
    avoids stt entirely and fuses decay+reset into one tensor_tensor via
    a {0, 0.5}-valued mask:
        A: M_t  = tt_add(M'_{t-1}, y_t)            int16, 2x   (y = x*4096)
        B: r2_t = ts(M_t is_lt 4096) mult 0.5      fp16 {0,.5}, 4x
        D: M'_t = tt_mult(M_t, r2_t)               int16, 2x = reset AND decay
    (DMA-CCE accumulate was tried for A: exact for int16, but its RMW
    doubles SDMA port traffic and its ~5.5 us round-trip needs >=4
    in-flight chunk chains to hide, which PSUM (8 banks = 2 chunks of
    packing state) cannot support — measured 127-263 us. Pure-DVE wins.)
  * Packed u8 output (32x less output traffic): PE accumulates 8 matmuls
    with diagonal fp16 weights -2^(t+1) over the r2 planes into PSUM:
    psum = -sum_t 2^t r_t = packed_byte - 255; ACT adds 255 during the
    PSUM -> SBUF u8 copy. byte bit t = spike at step t.
  * Per-core HBM traffic: 16 MiB in + 1 MiB out (~50 us at 358 GB/s).
  * Uneven chunks [4096, 2048, 2048]: the big chunks amortize DVE
    per-instruction overhead; the final 2048 chunk halves the kernel tail
    (last mask -> 4 matmuls -> copies -> out-DMA). Input arrives as
    per-timestep 0.5-1 MiB DMAs, and the t=0/t=1 strips of chunk 0 are
    issued first so compute starts ~3 us in.

Per-core layout: data-parallel over the leading dim (64 -> 8 per core),
t-major strips [128 partitions, T=8, 8192 neurons]; all compute touches
contiguous strips (keeps DVE 2x/4x modes + dense DMA descriptors).
"""

import numpy as np

import concourse.bass as bass
import concourse.tile as tile
from concourse import bacc, mybir
from concourse.bass_utils import run_bass_kernel_spmd

P = 128          # SBUF partitions
T = 8            # timesteps (innermost axis of the original input)
NPB = 8192       # neurons per partition per core: 8*128*32*32 / 128
CHUNKS = (4096, 4096)
PSB = 512        # psum bank free size (fp32)

SCALE = 4096.0   # fixed-point scale 2^12
THR = 4096.0     # threshold 1.0 in scaled units
N_CORES = 8

F32 = mybir.dt.float32
I16 = mybir.dt.int16
U8 = mybir.dt.uint8
F16 = mybir.dt.float16

Alu = mybir.AluOpType
Act = mybir.ActivationFunctionType


def _build() -> bass.Bass:
    nc = bacc.Bacc("TRN2", target_bir_lowering=False, debug=False)
    x = nc.dram_tensor("x", [P, T, NPB], I16, kind="ExternalInput").ap()
    w = nc.dram_tensor("w", [P, T * P], F16, kind="ExternalInput").ap()
    y = nc.dram_tensor("y", [P, NPB], U8, kind="ExternalOutput").ap()

    with tile.TileContext(nc) as tc:
        with (
            tc.tile_pool(name="xin", bufs=10) as xin,
            tc.tile_pool(name="mem", bufs=6) as mem,
            tc.tile_pool(name="msk", bufs=6) as msk,
            tc.tile_pool(name="acc", bufs=2) as accp,
            tc.tile_pool(name="wts", bufs=1) as wts,
            tc.tile_pool(name="ps", bufs=1, space="PSUM") as psp,
        ):
            wt = wts.tile([P, T * P], F16, tag="w", name="wt")
            nc.sync.dma_start(wt[:], w[:, :])
            bias_p = wts.tile([P, 1], F32, tag="bp", name="bias_p")
            nc.vector.memset(bias_p[:], 255.0)

            lo = 0
            bank0 = 0
            for c, CH in enumerate(CHUNKS):
                NB = CH // PSB
                xs = []
                for t in range(T):
                    st = xin.tile([P, CH], I16, tag="x", name=f"x{c}_{t}")
                    nc.sync.dma_start(st[:], x[:, t, lo : lo + CH])
                    xs.append(st[:])
                ps = [
                    psp.tile(
                        [P, PSB], F32, tag=f"ps{(bank0 + b) % 8}",
                        name=f"ps{c}_{b}",
                    )
                    for b in range(NB)
                ]
                acc = accp.tile([P, CH], U8, tag="a", name=f"a{c}")
                cur = xs[0]  # M_0 = y_0 (mem starts at 0): alias, no copy
                for t in range(T):
                    if t > 0:
                        nxt = mem.tile([P, CH], I16, tag="m", name=f"m{c}_{t}")
                        nc.vector.tensor_tensor(nxt[:], prev[:], xs[t], Alu.add)
                        cur = nxt[:]
                    r2 = msk.tile([P, CH], F16, tag="r", name=f"r{c}_{t}")
                    nc.vector.tensor_scalar(
                        r2[:], cur, THR, 0.5, Alu.is_lt, Alu.mult
                    )
                    for b in range(NB):
                        nc.tensor.matmul(
                            ps[b][:],
                            wt[:, t * P : (t + 1) * P],
                            r2[:, b * PSB : (b + 1) * PSB],
                            start=(t == 0),
                            stop=(t == T - 1),
                        )
                    if t < T - 1:
                        rst = mem.tile([P, CH], I16, tag="m", name=f"d{c}_{t}")
                        nc.vector.tensor_tensor(rst[:], cur, r2[:], Alu.mult)
                        prev = rst
                for b in range(NB):
                    nc.scalar.activation(
                        acc[:, b * PSB : (b + 1) * PSB],
                        ps[b][:],
                        Act.Identity,
                        bias=bias_p[:],
                        scale=1.0,
                    )
                    if b % 4 == 3:
                        nc.sync.dma_start(
                            y[:, lo + (b - 3) * PSB : lo + (b + 1) * PSB],
                            acc[:, (b - 3) * PSB : (b + 1) * PSB],
                        )
                lo += CH
                bank0 += NB
    nc.compile()
    return nc


_NC_CACHE: bass.Bass | None = None


def _get_nc() -> bass.Bass:
    global _NC_CACHE
    if _NC_CACHE is None:
        _NC_CACHE = _build()
    return _NC_CACHE


def _weights() -> np.ndarray:
    # W_t = -2^(t+1) * I, laid out as [P, T*P] (lhsT slices [128, 128] per t).
    # psum = sum_t W_t^T r2_t = -sum_t 2^t r_t = packed_byte - 255.
    wf = np.zeros((P, T * P), dtype=np.float32)
    for t in range(T):
        wf[:, t * P : (t + 1) * P][np.arange(P), np.arange(P)] = -(2.0 ** (t + 1))
    return wf.astype(np.float16)


def _run(X: np.ndarray, **spmd_kwargs):
    assert X.shape == (64, 128, 32, 32, 8), X.shape
    X = np.asarray(X, dtype=np.float32)
    per_core = 64 // N_CORES
    q = np.clip(np.rint(X * SCALE), -32768.0, 32767.0).astype(np.int16)
    # [core, p, n, t] -> t-major [core, p, t, n], contiguous per core
    qt = np.ascontiguousarray(
        q.reshape(N_CORES, P, NPB, T).transpose(0, 1, 3, 2)
    )
    wnp = _weights()
    in_maps = [{"x": qt[i], "w": wnp} for i in range(N_CORES)]
    res = run_bass_kernel_spmd(
        _get_nc(), in_maps, core_ids=list(range(N_CORES)), **spmd_kwargs
    )
    out = np.empty_like(X)
    for i, r in enumerate(res.results):
        packed = r["y"].reshape(P, NPB, 1).astype(np.uint8)
        bits = np.unpackbits(packed, axis=2, bitorder="little")  # [P, NPB, 8]
        out[i * per_core : (i + 1) * per_core] = bits.astype(np.float32).reshape(
            per_core, 128, 32, 32, 8
        )
    return out, res


def kernel(X: np.ndarray) -> np.ndarray:
    out, _ = _run(X)
    return out


# revision 22
# speedup vs baseline: 1.0124x; 1.0068x over previous
"""LIF (leaky integrate-and-fire) scan kernel for Trainium2, 8 NeuronCores.

Reference semantics (fp32, T=8 innermost axis):
    mem = 0
    for t in range(T):
        mem = mem * 0.5 + x[..., t]
        s[..., t] = (mem >= 1.0)
        mem = mem * (1.0 - s[..., t])

Strategy (fp32 baseline ~218 us; v3 ~126 us; v4 ~109 us):
  * int16 fixed-point (scale 2^12): x quantized on the host, membrane M
    int16 on device. Engines compute fp32 internally, so the only error
    sources are x quantization and one int16 rounding of M per step
    (quantum 2^-12): 1841 spike flips vs the fp32 reference = rel err
    0.0137 < 2e-2 (robust to HW rounding mode; rne/trunc/floor/ceil all
    pass in host simulation).
  * HW-measured op selection (per 2048 elems/partition):
    scalar_tensor_tensor is ALWAYS 1x (2284 ns) regardless of dtype;
    tensor_tensor on 16-bit is 2x (1225 ns); tensor_scalar 16-bit in/out
    is 4x (694 ns); the mask must be fp16 (int16-out is_lt is
    pathological at 3529 ns, bf16 mixed-TT drops to 1628 ns). The update
    avoids stt entirely and fuses decay+reset into one tensor_tensor via
    a {0, 0.5}-valued mask:
        A: M_t  = tt_add(M'_{t-1}, y_t)            int16, 2x   (y = x*4096)
        B: r2_t = ts(M_t is_lt 4096) mult 0.5      fp16 {0,.5}, 4x
        D: M'_t = tt_mult(M_t, r2_t)               int16, 2x = reset AND decay
    (DMA-CCE accumulate was tried for A: exact for int16, but its RMW
    doubles SDMA port traffic and its ~5.5 us round-trip needs >=4
    in-flight chunk chains to hide, which PSUM (8 banks = 2 chunks of
    packing state) cannot support — measured 127-263 us. Pure-DVE wins.)
  * Packed u8 output (32x less output traffic): PE accumulates 8 matmuls
    with diagonal fp16 weights -2^(t+1) over the r2 planes into PSUM:
    psum = -sum_t 2^t r_t = packed_byte - 255; ACT adds 255 during the
    PSUM -> SBUF u8 copy. byte bit t = spike at step t.
  * Per-core HBM traffic: 16 MiB in + 1 MiB out (~50 us at 358 GB/s).
  * Uneven chunks [4096, 2048, 2048]: the big chunks amortize DVE
    per-instruction overhead; the final 2048 chunk halves the kernel tail
    (last mask -> 4 matmuls -> copies -> out-DMA). Input arrives as
    per-timestep 0.5-1 MiB DMAs, and the t=0/t=1 strips of chunk 0 are
    issued first so compute starts ~3 us in.

Per-core layout: data-parallel over the leading dim (64 -> 8 per core),
t-major strips [128 partitions, T=8, 8192 neurons]; all compute touches
contiguous strips (keeps DVE 2x/4x modes + dense DMA descriptors).
"""

import numpy as np

import concourse.bass as bass
import concourse.tile as tile
from concourse import bacc, mybir
from concourse.bass_utils import run_bass_kernel_spmd

P = 128          # SBUF partitions
T = 8            # timesteps (innermost axis of the original input)
NPB = 8192       # neurons per partition per core: 8*128*32*32 / 128
CHUNKS = (4096, 4096)
PSB = 512        # psum bank free size (fp32)

SCALE = 4096.0   # fixed-point scale 2^12
THR = 4096.0     # threshold 1.0 in scaled units
N_CORES = 8

F32 = mybir.dt.float32
I16 = mybir.dt.int16
U8 = mybir.dt.uint8
F16 = mybir.dt.float16

Alu = mybir.AluOpType
Act = mybir.ActivationFunctionType


def _build() -> bass.Bass:
    nc = bacc.Bacc("TRN2", target_bir_lowering=False, debug=False)
    x = nc.dram_tensor("x", [P, T, NPB], I16, kind="ExternalInput").ap()
    w = nc.dram_tensor("w", [P, T * P], F16, kind="ExternalInput").ap()
    y = nc.dram_tensor("y", [P, NPB], U8, kind="ExternalOutput").ap()

    with tile.TileContext(nc) as tc:
        with (
            tc.tile_pool(name="xin", bufs=9) as xin,
            tc.tile_pool(name="mem", bufs=6) as mem,
            tc.tile_pool(name="msk", bufs=6) as msk,
            tc.tile_pool(name="acc", bufs=2) as accp,
            tc.tile_pool(name="wts", bufs=1) as wts,
            tc.tile_pool(name="ps", bufs=1, space="PSUM") as psp,
        ):
            HH = 2048
            x0h = [
                xin.tile([P, HH], I16, tag="x0h", name=f"x0h{h}", bufs=2)
                for h in range(2)
            ]
            for h in range(2):
                nc.sync.dma_start(x0h[h][:], x[:, 0, h * HH : (h + 1) * HH])
            wt = wts.tile([P, T * P], F16, tag="w", name="wt")
            nc.sync.dma_start(wt[:], w[:, :])
            bias_p = wts.tile([P, 1], F32, tag="bp", name="bias_p")
            nc.vector.memset(bias_p[:], 255.0)

            lo = 0
            bank0 = 0
            for c, CH in enumerate(CHUNKS):
                NB = CH // PSB
                xs = []
                for t in range(T):
                    if c == 0 and t == 0:
                        xs.append(None)  # halves live in x0h
                        continue
                    st = xin.tile([P, CH], I16, tag="x", name=f"x{c}_{t}")
                    nc.sync.dma_start(st[:], x[:, t, lo : lo + CH])
                    xs.append(st[:])
                ps = [
                    psp.tile(
                        [P, PSB], F32, tag=f"ps{(bank0 + b) % 8}",
                        name=f"ps{c}_{b}",
                    )
                    for b in range(NB)
                ]
                acc = accp.tile([P, CH], U8, tag="a", name=f"a{c}")
                cur = xs[0]  # M_0 = y_0 (mem starts at 0): alias, no copy
                for t in range(T):
                    if c == 0 and t == 0:
                        # per-half mask+reset: each half starts the moment its
                        # 0.5 MiB strip half arrives
                        rst = mem.tile([P, CH], I16, tag="m", name="d0_0")
                        for h in range(2):
                            r2x = msk.tile(
                                [P, HH], F16, tag="rh", name=f"r0h{h}", bufs=2
                            )
                            nc.vector.tensor_scalar(
                                r2x[:], x0h[h][:], THR, 0.5, Alu.is_lt, Alu.mult
                            )
                            for b in range(4 * h, 4 * h + 4):
                                nc.tensor.matmul(
                                    ps[b][:],
                                    wt[:, 0:P],
                                    r2x[:, (b - 4 * h) * PSB : (b - 4 * h + 1) * PSB],
                                    start=True,
                                    stop=False,
                                )
                            nc.vector.tensor_tensor(
                                rst[:, h * HH : (h + 1) * HH],
                                x0h[h][:],
                                r2x[:],
                                Alu.mult,
                            )
                        prev = rst
                        continue
                    if t > 0:
                        nxt = mem.tile([P, CH], I16, tag="m", name=f"m{c}_{t}")
                        nc.vector.tensor_tensor(nxt[:], prev[:], xs[t], Alu.add)
                        cur = nxt[:]
                    r2 = msk.tile([P, CH], F16, tag="r", name=f"r{c}_{t}")
                    nc.vector.tensor_scalar(
                        r2[:], cur, THR, 0.5, Alu.is_lt, Alu.mult
                    )
                    for b in range(NB):
                        nc.tensor.matmul(
                            ps[b][:],
                            wt[:, t * P : (t + 1) * P],
                            r2[:, b * PSB : (b + 1) * PSB],
                            start=(t == 0),
                            stop=(t == T - 1),
                        )
                    if t < T - 1:
                        rst = mem.tile([P, CH], I16, tag="m", name=f"d{c}_{t}")
                        nc.vector.tensor_tensor(rst[:], cur, r2[:], Alu.mult)
                        prev = rst
                for b in range(NB):
                    nc.scalar.activation(
                        acc[:, b * PSB : (b + 1) * PSB],
                        ps[b][:],
                        Act.Identity,
                        bias=bias_p[:],
                        scale=1.0,
                    )
                    if b % 4 == 3:
                        nc.sync.dma_start(
                            y[:, lo + (b - 3) * PSB : lo + (b + 1) * PSB],
                            acc[:, (b - 3) * PSB : (b + 1) * PSB],
                        )
                lo += CH
                bank0 += NB
    nc.compile()
    return nc


_NC_CACHE: bass.Bass | None = None


def _get_nc() -> bass.Bass:
    global _NC_CACHE
    if _NC_CACHE is None:
        _NC_CACHE = _build()
    return _NC_CACHE


def _weights() -> np.ndarray:
    # W_t = -2^(t+1) * I, laid out as [P, T*P] (lhsT slices [128, 128] per t).
    # psum = sum_t W_t^T r2_t = -sum_t 2^t r_t = packed_byte - 255.
    wf = np.zeros((P, T * P), dtype=np.float32)
    for t in range(T):
        wf[:, t * P : (t + 1) * P][np.arange(P), np.arange(P)] = -(2.0 ** (t + 1))
    return wf.astype(np.float16)


def _run(X: np.ndarray, **spmd_kwargs):
    assert X.shape == (64, 128, 32, 32, 8), X.shape
    X = np.asarray(X, dtype=np.float32)
    per_core = 64 // N_CORES
    q = np.clip(np.rint(X * SCALE), -32768.0, 32767.0).astype(np.int16)
    # [core, p, n, t] -> t-major [core, p, t, n], contiguous per core
    qt = np.ascontiguousarray(
        q.reshape(N_CORES, P, NPB, T).transpose(0, 1, 3, 2)
    )
    wnp = _weights()
    in_maps = [{"x": qt[i], "w": wnp} for i in range(N_CORES)]
    res = run_bass_kernel_spmd(
        _get_nc(), in_maps, core_ids=list(range(N_CORES)), **spmd_kwargs
    )
    out = np.empty_like(X)
    for i, r in enumerate(res.results):
        packed = r["y"].reshape(P, NPB, 1).astype(np.uint8)
        bits = np.unpackbits(packed, axis=2, bitorder="little")  # [P, NPB, 8]
        out[i * per_core : (i + 1) * per_core] = bits.astype(np.float32).reshape(
            per_core, 128, 32, 32, 8
        )
    return out, res


def kernel(X: np.ndarray) -> np.ndarray:
    out, _ = _run(X)
    return out
